# revision 1
# baseline (speedup 1.0000x reference)
"""Trainium2 Bass kernel for nn_CombinedPairwiseCacheLoss.

Computes, on 8 NeuronCores, the circle-style pairwise cache loss:
    emb_n = l2norm(embedding)                       # [N, D]
    cache = concat(emb_n, old_cache_features)[:M]   # [M, D]
    dist  = emb_n @ cache.T                         # [N, M]
    ... masked positive/negative logits, per-row logsumexp, softplus, mean.

Sharding: the cache (M=10000 rows) is split column-wise into 8 slabs of 1250
(padded to 1280).  Each core computes its local GEMM tile [1024 x 1280] plus
local masked sum-exp partials (fixed-offset logsumexp, so cross-core combine
is a plain sum done on the host during the gather step).

Device math per element (d = cosine similarity, m = label-match mask in {0,1}):
    sum_n partial:  exp(30*d^2       - 30*m      - 30  )   # == exp(l_n - 25.2)
    sum_p partial:  exp(30*(d-1)^2   - 30*(1-m)  - 44.8)   # == exp(l_p - 40.0)
The m=0/1 mask gives the wrong-side entries an extra e^-30 suppression factor,
which is far below the 1e-5-level accuracy of everything else (validated
against the reference in f64).  Host: lse_n = 25.2 + log(sum_n),
lse_p = 40 + log(sum_p) after subtracting the analytically-known diagonal and
zero-pad contributions, then mean(softplus(lse_p + lse_n)).

Distance matmuls run in float32r (full-rate PE, ~19-bit mantissa), which
lands the final loss within ~5e-7 relative of the f32 reference.
"""

import os
import sys

for _p in ("/opt/trn_rl_repo", "/root/.axon_site/_ro/trn_rl_repo"):
    if os.path.isdir(_p) and _p not in sys.path:
        sys.path.insert(0, _p)

import numpy as np

import concourse.bacc as bacc
import concourse.tile as tile
from concourse import mybir
from concourse.bass_utils import run_bass_kernel_spmd

F32 = mybir.dt.float32
F32R = mybir.dt.float32r
AF = mybir.ActivationFunctionType
ALU = mybir.AluOpType

NCORES = 8
N = 1024
D = 1024
M = 10000
SLAB = 1250          # cache rows per core
SLABP = 1280         # padded to a multiple of 128
NPAD = SLABP - SLAB  # 30 zero-padded cache rows per core
JCHUNKS = [(0, 512), (512, 512), (1024, 256)]  # bank-aligned psum regions
NB_I = 8             # 1024 rows / 128

USE_F32R = True

_NC_CACHE = {}


def _build_nc(use_f32r=USE_F32R):
    nc = bacc.Bacc(
        "TRN2", target_bir_lowering=False, debug=False, num_devices=NCORES
    )
    MDT = F32R if use_f32r else F32
    embT = nc.dram_tensor("embT", [D, N], MDT, kind="ExternalInput").ap()
    slabT = nc.dram_tensor("slabT", [D, SLABP], MDT, kind="ExternalInput").ap()
    labB = nc.dram_tensor("labB", [128, SLABP], F32, kind="ExternalInput").ap()
    tgtC = nc.dram_tensor("tgtC", [128, NB_I], F32, kind="ExternalInput").ap()
    pselC = nc.dram_tensor("pselC", [128, NB_I], F32, kind="ExternalInput").ap()
    ident = nc.dram_tensor("ident", [128, 128], F32, kind="ExternalInput").ap()
    onesI = nc.dram_tensor("onesI", [128, 128], MDT, kind="ExternalInput").ap()
    out = nc.dram_tensor("out", [2, 128, NB_I], F32, kind="ExternalOutput").ap()

    def f32view(ap):
        return ap.bitcast(F32) if use_f32r else ap

    with tile.TileContext(nc) as tc:
        with (
            tc.tile_pool(name="persist", bufs=1) as P,
            tc.tile_pool(name="emb", bufs=1) as PEmb,
            tc.tile_pool(name="slab", bufs=1) as PSlab,
            tc.tile_pool(name="sqn", bufs=2) as Psq,
            tc.tile_pool(name="work", bufs=2) as W,
            tc.tile_pool(name="psum_d", bufs=2, space="PSUM") as PP,
            tc.tile_pool(name="psum_s", bufs=2, space="PSUM") as PPs,
        ):
            # constants
            biasn = P.tile([128, 1], F32)
            nc.vector.memset(biasn[:], -30.0)
            biasp = P.tile([128, 1], F32)
            nc.vector.memset(biasp[:], -44.8)
            neg1 = P.tile([128, 1], F32)
            nc.vector.memset(neg1[:], -1.0)
            scratch1 = P.tile([128, 1], F32)
            # dummy activations: pull the Square/Exp/Sqrt LUT loads off the
            # critical path (each costs ~1.3us on first use)
            nc.scalar.activation(scratch1[:], biasn[:], AF.Square)
            nc.scalar.activation(scratch1[:], biasn[:], AF.Exp)
            nc.scalar.activation(scratch1[:], scratch1[:], AF.Sqrt)

            # inputs — two DMA queues: embT + labB on HWDGE/sync,
            # slab + small tensors on SWDGE/gpsimd.
            ones = P.tile([128, 128], MDT)
            nc.gpsimd.dma_start(ones[:], onesI[:])
            tgt_sb = P.tile([128, NB_I], F32)
            nc.gpsimd.dma_start(tgt_sb[:], tgtC[:])
            psel_sb = P.tile([128, NB_I], F32)
            nc.gpsimd.dma_start(psel_sb[:], pselC[:])
            id_sb = P.tile([128, 128], F32)
            nc.gpsimd.dma_start(id_sb[:], ident[:])

            embT_sb = []
            for dd in range(8):
                t = PEmb.tile([128, N], MDT, name=f"embT{dd}", tag=f"embT{dd}")
                nc.sync.dma_start(t[:], embT[dd * 128 : (dd + 1) * 128, :])
                embT_sb.append(t)
            labB_sb = P.tile([128, SLABP], F32)
            nc.sync.dma_start(labB_sb[:], labB[:])
            slab_sb = []
            for dd in range(8):
                t = PSlab.tile([128, SLABP], MDT, name=f"slab{dd}", tag=f"slab{dd}")
                nc.gpsimd.dma_start(t[:], slabT[dd * 128 : (dd + 1) * 128, :])
                slab_sb.append(t)

            # ---- embedding row norms:  norms2[i] = sum_dd embT[dd, i]^2
            ps_norm = [
                PPs.tile([1, 512], F32, name=f"psn{h}", tag="pss") for h in range(2)
            ]
            for dd in range(8):
                sq = Psq.tile([128, N], MDT, name="sq", tag="sqn")
                if dd % 2 == 0:
                    nc.vector.tensor_mul(
                        sq[:], f32view(embT_sb[dd][:]), f32view(embT_sb[dd][:])
                    )
                else:
                    nc.scalar.activation(sq[:], f32view(embT_sb[dd][:]), AF.Square)
                for h in range(2):
                    nc.tensor.matmul(
                        ps_norm[h][:],
                        ones[:, 0:1],
                        sq[:, h * 512 : (h + 1) * 512],
                        start=(dd == 0),
                        stop=(dd == 7),
                    )
            n2_free = P.tile([1, N], MDT)
            for h in range(2):
                nc.scalar.copy(n2_free[0:1, h * 512 : (h + 1) * 512], ps_norm[h][:])

            # transpose norms2 into per-partition column layout [128, 8]
            ps_nc = PPs.tile([128, NB_I], F32, name="psnc", tag="pss")
            for ib in range(NB_I):
                nc.tensor.matmul(
                    ps_nc[:, ib : ib + 1],
                    f32view(n2_free[0:1, ib * 128 : (ib + 1) * 128]),
                    f32view(ones[0:1, 0:1]),
                    start=True,
                    stop=True,
                )
            n2_col = P.tile([128, NB_I], F32)
            nc.scalar.copy(n2_col[:], ps_nc[:])
            inv2 = P.tile([128, NB_I], F32)
            nc.vector.reciprocal(inv2[:], n2_col[:])
            rinv = P.tile([128, NB_I], F32)
            nc.scalar.activation(rinv[:], inv2[:], AF.Sqrt)

            # scol = psel * (rinv - 1) + 1  (per-core column scale for the raw
            # embedding block inside core 0's cache slab; identity elsewhere)
            sc0 = P.tile([128, NB_I], F32)
            nc.vector.tensor_scalar(sc0[:], rinv[:], -1.0, None, ALU.add)
            sc1 = P.tile([128, NB_I], F32)
            nc.vector.tensor_mul(sc1[:], sc0[:], psel_sb[:])
            scol_c = P.tile([128, NB_I], F32)
            nc.vector.tensor_scalar(scol_c[:], sc1[:], 1.0, None, ALU.add)

            # transpose [128, 8] columns into a [1, 1024] free-layout row:
            # scol_c[:, b].T @ I gives row b*128..(b+1)*128
            scol_free = P.tile([1, N], MDT)
            for h in range(2):
                ps_f = PPs.tile([1, 512], F32, name=f"psf{h}", tag="pss")
                for bb in range(4):
                    b = h * 4 + bb
                    nc.tensor.matmul(
                        ps_f[0:1, bb * 128 : (bb + 1) * 128],
                        scol_c[:, b : b + 1],
                        id_sb[:],
                        start=True,
                        stop=True,
                    )
                nc.scalar.copy(scol_free[0:1, h * 512 : (h + 1) * 512], ps_f[:])

            # broadcast scol [1, 1024] -> [128, 1024]
            scolB = P.tile([128, N], F32)
            for h in range(2):
                ps_b = PPs.tile([128, 512], F32, name=f"psb{h}", tag="pss")
                nc.tensor.matmul(
                    ps_b[:],
                    ones[0:1, :],
                    scol_free[0:1, h * 512 : (h + 1) * 512],
                    start=True,
                    stop=True,
                )
                nc.scalar.copy(scolB[:, h * 512 : (h + 1) * 512], ps_b[:])

            # scale the raw-embedding block of the cache slab (cols 0..1023)
            for dd in range(8):
                nc.vector.tensor_mul(
                    slab_sb[dd][:, 0:N], f32view(slab_sb[dd][:, 0:N]), scolB[:]
                )

            # ---- main loop: one 3-bank psum tile [128, 1280] per row block,
            # whole-width epilogue (one instruction per stage).
            acc_n = P.tile([128, NB_I], F32)
            acc_p = P.tile([128, NB_I], F32)
            for ib in range(NB_I):
                rinv_ib = rinv[:, ib : ib + 1]
                tgt_ib = tgt_sb[:, ib : ib + 1]
                ps_d = PP.tile([128, SLABP], F32, name="psd", tag="psd")
                for j0, jw in JCHUNKS:
                    for dd in range(8):
                        nc.tensor.matmul(
                            ps_d[:, j0 : j0 + jw],
                            embT_sb[dd][:, ib * 128 : (ib + 1) * 128],
                            slab_sb[dd][:, j0 : j0 + jw],
                            start=(dd == 0),
                            stop=(dd == 7),
                        )
                # q = (rinv*g)^2 ;  s2 = (rinv*g - 1)^2
                q = W.tile([128, SLABP], F32, name="q", tag="q")
                nc.scalar.activation(
                    q[:], ps_d[:], AF.Square, bias=0.0, scale=rinv_ib
                )
                s2 = W.tile([128, SLABP], F32, name="s2", tag="s2")
                nc.scalar.activation(
                    s2[:], ps_d[:], AF.Square, bias=neg1[:, 0:1], scale=rinv_ib
                )
                # zn = (lab == tgt) - q ; zp = (lab != tgt) - s2
                zn = W.tile([128, SLABP], F32, name="zn", tag="zn")
                nc.vector.scalar_tensor_tensor(
                    zn[:], labB_sb[:], tgt_ib, q[:], ALU.is_equal, ALU.subtract
                )
                zp = W.tile([128, SLABP], F32, name="zp", tag="zp")
                nc.vector.scalar_tensor_tensor(
                    zp[:], labB_sb[:], tgt_ib, s2[:], ALU.not_equal, ALU.subtract
                )
                # en = exp(-30*zn - 30) ; ep = exp(-30*zp - 44.8)
                en = W.tile([128, SLABP], F32, name="en", tag="en")
                nc.scalar.activation(
                    en[:],
                    zn[:],
                    AF.Exp,
                    bias=biasn[:, 0:1],
                    scale=-30.0,
                    accum_out=acc_n[:, ib : ib + 1],
                )
                ep = W.tile([128, SLABP], F32, name="ep", tag="ep")
                nc.scalar.activation(
                    ep[:],
                    zp[:],
                    AF.Exp,
                    bias=biasp[:, 0:1],
                    scale=-30.0,
                    accum_out=acc_p[:, ib : ib + 1],
                )

            nc.sync.dma_start(out[0, :, :], acc_n[:])
            nc.sync.dma_start(out[1, :, :], acc_p[:])

    nc.compile()
    return nc


def _get_nc():
    key = USE_F32R
    if key not in _NC_CACHE:
        _NC_CACHE[key] = _build_nc(key)
    return _NC_CACHE[key]


def _prepare_in_maps(embedding, old_cache_features, targets, old_cache_labels):
    emb = np.ascontiguousarray(np.asarray(embedding, dtype=np.float32))
    oc = np.ascontiguousarray(np.asarray(old_cache_features, dtype=np.float32))
    tg = np.asarray(targets).astype(np.float64)
    ol = np.asarray(old_cache_labels).astype(np.float64)
    cache_labels = np.concatenate([tg, ol])[:M]

    embT = np.ascontiguousarray(emb.T)
    ident = np.eye(128, dtype=np.float32)
    ones_arr = np.ones((128, 128), dtype=np.float32)
    tgtC = np.ascontiguousarray(tg.reshape(NB_I, 128).T.astype(np.float32))

    in_maps = []
    for k in range(NCORES):
        j0 = SLAB * k
        if k == 0:
            rows = np.concatenate([emb, oc[0 : SLAB - N]], axis=0)
        else:
            rows = oc[j0 - N : j0 - N + SLAB]
        slabT = np.zeros((D, SLABP), np.float32)
        slabT[:, :SLAB] = rows.T
        labs = np.full(SLABP, -1.0, np.float64)
        labs[:SLAB] = cache_labels[j0 : j0 + SLAB]
        labB = np.ascontiguousarray(
            np.broadcast_to(labs.astype(np.float32), (128, SLABP))
        )
        pselC = np.full((128, NB_I), 1.0 if k == 0 else 0.0, np.float32)
        in_maps.append(
            dict(
                embT=embT,
                slabT=slabT,
                labB=labB,
                tgtC=tgtC,
                pselC=pselC,
                ident=ident,
                onesI=ones_arr,
            )
        )
    return in_maps


def _postprocess(results):
    sn = np.zeros(N, np.float64)
    sp = np.zeros(N, np.float64)
    for k in range(NCORES):
        o = np.asarray(results[k]["out"], np.float64)  # [2, 128, 8]
        sn += o[0].T.reshape(N)
        sp += o[1].T.reshape(N)
    # Analytic corrections (see module docstring):
    #  - the self-match (diagonal) term appears once per row on core 0:
    #    exp(-30) in sum_n (label matches, m=1) and exp(-44.8) in sum_p.
    #  - each of the 8*30 zero-padded cache rows contributes exp(-30) to
    #    sum_n (label -1 never matches, d=0) and exp(-44.8) to sum_p.
    sn -= (1 + NCORES * NPAD) * np.exp(-30.0)
    sp -= (1 + NCORES * NPAD) * np.exp(-44.8)
    lse_n = 25.2 + np.log(np.maximum(sn, 1e-300))
    lse_p = 40.0 + np.log(np.maximum(sp, 1e-300))
    loss = np.mean(np.logaddexp(0.0, lse_p + lse_n))
    return np.float32(loss)


def _run(in_maps, trace=False, **kwargs):
    nc = _get_nc()
    return run_bass_kernel_spmd(
        nc, in_maps, core_ids=list(range(NCORES)), trace=trace, **kwargs
    )


def kernel(embedding, old_cache_features, targets, old_cache_labels):
    in_maps = _prepare_in_maps(
        embedding, old_cache_features, targets, old_cache_labels
    )
    res = _run(in_maps)
    return _postprocess(res.results)



# revision 3
# speedup vs baseline: 1.7115x; 1.7115x over previous
"""Trainium2 Bass kernel for nn_CombinedPairwiseCacheLoss (v2).

Math (d = cosine similarity, m = label-match mask in {0,1}):
    loss = mean(softplus(lse_p + lse_n))
    lse_n = logsumexp_j(30*d^2 - 4.8)   over negatives (m=0)
    lse_p = logsumexp_j(30*(d-1)^2 - 4.8) over positives (m=1, minus diagonal)

Device trick (plan V): fold the mask INTO the quadratic so each side needs
only one masked stt + one square + one exp:
    un = (1-m)*d        ->  sum_n_dev = sum_j exp(30*un^2 - 30)
                            (m=1 entries contribute exactly exp(-30) each;
                             host subtracts cnt_i*exp(-30), cnt from bincount)
    v  = m - d          ->  sum_p_dev = sum_j exp(30*v^2  - 44.8)
                            (m=0 entries are suppressed by the quadratic
                             itself, ~1e-10 relative; host subtracts the
                             known diagonal term)
    lse_n = 25.2 + log(sum_n), lse_p = 40 + log(sum_p).

Embedding is l2-normalized on the host (0.02% of total FLOPs); each core's
cache slab (1250 rows, padded to 1280) is fully prepared host-side, so the
device program is just: DMA in -> GEMM -> 4 cheap elementwise ops + 2 exps
per 128-row block -> partial-sum DMA out.

Variants:
  "bf16":  [D=1024] contraction as 8 k-planes of 128, bf16 operands.
  "fp8dr": fp8 e4m3 operands, DoubleRow perf mode (2 k-planes per matmul,
           2 MACs/cell/cycle) -> ~2x tensor-engine throughput.
Validated in numpy vs the f64 reference: rel err ~3e-5 for both (incl. the
fp16 epilogue intermediates); gate is 2e-2.
"""

import math
import os
import sys

for _p in ("/opt/trn_rl_repo", "/root/.axon_site/_ro/trn_rl_repo"):
    if os.path.isdir(_p) and _p not in sys.path:
        sys.path.insert(0, _p)

import numpy as np
import ml_dtypes

import concourse.bacc as bacc
import concourse.tile as tile
from concourse import mybir
from concourse.bass_utils import run_bass_kernel_spmd

F32 = mybir.dt.float32
FP16 = mybir.dt.float16
AF = mybir.ActivationFunctionType
ALU = mybir.AluOpType

NCORES = 8
N = 1024
D = 1024
M = 10000
SLAB = 1250
SLABP = 1280
NPAD = SLABP - SLAB
JCHUNKS = [(0, 512), (512, 512), (1024, 256)]
NB_I = 8
SQRT30 = math.sqrt(30.0)

VARIANT = "fp8dr"  # "bf16" | "fp8dr"

_NC_CACHE = {}


def _build_nc(variant):
    nc = bacc.Bacc(
        "TRN2", target_bir_lowering=False, debug=False, num_devices=NCORES
    )
    DT = mybir.dt.float8e4 if variant == "fp8dr" else mybir.dt.bfloat16

    embD = nc.dram_tensor("embD", [128, 8 * 1024], DT, kind="ExternalInput").ap()
    slabD = nc.dram_tensor("slabD", [128, 8 * SLABP], DT, kind="ExternalInput").ap()
    labD = nc.dram_tensor("labD", [128, SLABP], FP16, kind="ExternalInput").ap()
    tgtD = nc.dram_tensor("tgtD", [128, NB_I], FP16, kind="ExternalInput").ap()
    out = nc.dram_tensor("out", [2, 128, NB_I], F32, kind="ExternalOutput").ap()

    with tile.TileContext(nc) as tc:
        with (
            tc.tile_pool(name="persist", bufs=1) as P,
            tc.tile_pool(name="emb", bufs=1) as PE,
            tc.tile_pool(name="slab", bufs=1) as PS,
            tc.tile_pool(name="work", bufs=3) as W,
            tc.tile_pool(name="psum_d", bufs=2, space="PSUM") as PP,
        ):
            biasn = P.tile([128, 1], F32)
            nc.vector.memset(biasn[:], -30.0)
            biasp = P.tile([128, 1], F32)
            nc.vector.memset(biasp[:], -44.8)
            scratch = P.tile([128, 1], F32)
            # pull the Exp/Square LUT loads off the critical path
            nc.scalar.activation(scratch[:], biasn[:], AF.Exp)
            nc.scalar.activation(scratch[:], scratch[:], AF.Square)

            labB = P.tile([128, SLABP], FP16)
            nc.scalar.dma_start(labB[:], labD[:])
            tgt_sb = P.tile([128, NB_I], FP16)
            nc.scalar.dma_start(tgt_sb[:], tgtD[:])

            embS = PE.tile([128, 8, 1024], DT)
            slabS = PS.tile([128, 8, SLABP], DT)
            # k-plane-major arrival order so the first matmuls start early
            for s in range(8):
                eq = nc.sync if s % 2 == 0 else nc.scalar
                eq.dma_start(embS[:, s, :], embD[:, s * 1024 : (s + 1) * 1024])
                nc.gpsimd.dma_start(
                    slabS[:, s, :], slabD[:, s * SLABP : (s + 1) * SLABP]
                )

            acc_n = P.tile([128, NB_I], F32)
            acc_p = P.tile([128, NB_I], F32)
            for ib in range(NB_I):
                i0 = ib * 128
                ps = PP.tile([128, 1536], F32, name="ps", tag="ps")
                if variant == "fp8dr":
                    for t in range(4):
                        lhs = embS[:, 2 * t : 2 * t + 2, i0 : i0 + 128]
                        for j0, jw in JCHUNKS:
                            nc.tensor.matmul(
                                ps[:, j0 : j0 + jw],
                                lhs,
                                slabS[:, 2 * t : 2 * t + 2, j0 : j0 + jw],
                                start=(t == 0),
                                stop=(t == 3),
                                perf_mode=mybir.MatmulPerfMode.DoubleRow,
                            )
                else:
                    for dd in range(8):
                        lhs = embS[:, dd, i0 : i0 + 128]
                        for j0, jw in JCHUNKS:
                            nc.tensor.matmul(
                                ps[:, j0 : j0 + jw],
                                lhs,
                                slabS[:, dd, j0 : j0 + jw],
                                start=(dd == 0),
                                stop=(dd == 7),
                            )
                g = ps[:, 0:SLABP]
                tgt_ib = tgt_sb[:, ib : ib + 1]
                # v = m - g ; un = (1-m)*g      (DVE, psum-source)
                v16 = W.tile([128, SLABP], FP16, name="v16", tag="v16")
                nc.vector.scalar_tensor_tensor(
                    v16[:], labB[:], tgt_ib, g, ALU.is_equal, ALU.subtract
                )
                un16 = W.tile([128, SLABP], FP16, name="un16", tag="un16")
                nc.vector.scalar_tensor_tensor(
                    un16[:], labB[:], tgt_ib, g, ALU.not_equal, ALU.mult
                )
                # vsq = v*v (DVE fp16 2x) ; usq = 30*un^2 (ACT Square)
                vsq = W.tile([128, SLABP], FP16, name="vsq", tag="vsq")
                nc.vector.tensor_mul(vsq[:], v16[:], v16[:])
                usq = W.tile([128, SLABP], FP16, name="usq", tag="usq")
                nc.scalar.activation(usq[:], un16[:], AF.Square, scale=SQRT30)
                # en = exp(usq - 30) ; ep = exp(30*vsq - 44.8)   (ACT, accum)
                en16 = W.tile([128, SLABP], FP16, name="en16", tag="en16")
                nc.scalar.activation(
                    en16[:],
                    usq[:],
                    AF.Exp,
                    bias=biasn[:, 0:1],
                    scale=1.0,
                    accum_out=acc_n[:, ib : ib + 1],
                )
                ep16 = W.tile([128, SLABP], FP16, name="ep16", tag="ep16")
                nc.scalar.activation(
                    ep16[:],
                    vsq[:],
                    AF.Exp,
                    bias=biasp[:, 0:1],
                    scale=30.0,
                    accum_out=acc_p[:, ib : ib + 1],
                )

            nc.sync.dma_start(out[0, :, :], acc_n[:])
            nc.sync.dma_start(out[1, :, :], acc_p[:])

    nc.compile()
    return nc


def _get_nc(variant=None):
    variant = variant or VARIANT
    if variant not in _NC_CACHE:
        _NC_CACHE[variant] = _build_nc(variant)
    return _NC_CACHE[variant]


def _prepare(embedding, old_cache_features, targets, old_cache_labels, variant=None):
    variant = variant or VARIANT
    np_dt = ml_dtypes.float8_e4m3 if variant == "fp8dr" else ml_dtypes.bfloat16

    emb = np.asarray(embedding, np.float32)
    oc = np.asarray(old_cache_features, np.float32)
    tg = np.asarray(targets, np.int64)
    ol = np.asarray(old_cache_labels, np.int64)

    embn = emb / np.linalg.norm(emb, axis=1, keepdims=True)
    cache = np.concatenate([embn, oc])[:M]
    labels = np.concatenate([tg, ol])[:M]

    cache_q = cache.astype(np_dt)
    embn_q = embn.astype(np_dt)
    # [128, 8, 1024] k-plane-major layout of embn.T
    embD = np.ascontiguousarray(
        embn_q.T.reshape(8, 128, N).transpose(1, 0, 2).reshape(128, 8 * N)
    )

    tgtC = np.ascontiguousarray(
        tg.reshape(NB_I, 128).T.astype(np.float16)
    )

    in_maps = []
    for k in range(NCORES):
        rows = cache_q[SLAB * k : SLAB * k + SLAB]  # [1250, D] quantized
        slabT = np.zeros((D, SLABP), np_dt)
        slabT[:, :SLAB] = rows.T
        slabD = np.ascontiguousarray(
            slabT.reshape(8, 128, SLABP).transpose(1, 0, 2).reshape(128, 8 * SLABP)
        )
        labs = np.full(SLABP, -1.0, np.float32)
        labs[:SLAB] = labels[SLAB * k : SLAB * k + SLAB]
        labB = np.ascontiguousarray(
            np.broadcast_to(labs.astype(np.float16), (128, SLABP))
        )
        in_maps.append(dict(embD=embD, slabD=slabD, labD=labB, tgtD=tgtC))

    # host-side corrections
    lab_counts = np.bincount(labels, minlength=1000)
    cnt = lab_counts[tg]  # per-row label matches incl. the diagonal
    gii = np.sum(embn_q.astype(np.float64) ** 2, axis=1)  # quantized diag sim
    aux = dict(cnt=cnt.astype(np.float64), gii=gii)
    return in_maps, aux


def _post(results, aux):
    sn = np.zeros(N, np.float64)
    sp = np.zeros(N, np.float64)
    for k in range(NCORES):
        o = np.asarray(results[k]["out"], np.float64)  # [2, 128, 8]
        sn += o[0].T.reshape(N)
        sp += o[1].T.reshape(N)
    sn -= (aux["cnt"] + NCORES * NPAD) * np.exp(-30.0)
    sp -= np.exp(30.0 * (1.0 - aux["gii"]) ** 2 - 44.8) + NCORES * NPAD * np.exp(
        -44.8
    )
    lse_n = 25.2 + np.log(np.maximum(sn, 1e-300))
    lse_p = 40.0 + np.log(np.maximum(sp, 1e-300))
    loss = np.mean(np.logaddexp(0.0, lse_p + lse_n))
    return np.float32(loss)


def _run(in_maps, variant=None, trace=False, **kwargs):
    nc = _get_nc(variant)
    return run_bass_kernel_spmd(
        nc, in_maps, core_ids=list(range(NCORES)), trace=trace, **kwargs
    )


def kernel(embedding, old_cache_features, targets, old_cache_labels):
    in_maps, aux = _prepare(
        embedding, old_cache_features, targets, old_cache_labels
    )
    res = _run(in_maps)
    return _post(res.results, aux)


# revision 5
# speedup vs baseline: 1.9291x; 1.1272x over previous
"""Trainium2 Bass kernel for nn_CombinedPairwiseCacheLoss (v2).

Math (d = cosine similarity, m = label-match mask in {0,1}):
    loss = mean(softplus(lse_p + lse_n))
    lse_n = logsumexp_j(30*d^2 - 4.8)   over negatives (m=0)
    lse_p = logsumexp_j(30*(d-1)^2 - 4.8) over positives (m=1, minus diagonal)

Device trick (plan V): fold the mask INTO the quadratic so each side needs
only one masked stt + one square + one exp:
    un = (1-m)*d        ->  sum_n_dev = sum_j exp(30*un^2 - 30)
                            (m=1 entries contribute exactly exp(-30) each;
                             host subtracts cnt_i*exp(-30), cnt from bincount)
    v  = m - d          ->  sum_p_dev = sum_j exp(30*v^2  - 44.8)
                            (m=0 entries are suppressed by the quadratic
                             itself, ~1e-10 relative; host subtracts the
                             known diagonal term)
    lse_n = 25.2 + log(sum_n), lse_p = 40 + log(sum_p).

Embedding is l2-normalized on the host (0.02% of total FLOPs); each core's
cache slab (1250 rows, padded to 1280) is fully prepared host-side, so the
device program is just: DMA in -> GEMM -> 4 cheap elementwise ops + 2 exps
per 128-row block -> partial-sum DMA out.

Variants:
  "bf16":  [D=1024] contraction as 8 k-planes of 128, bf16 operands.
  "fp8dr": fp8 e4m3 operands, DoubleRow perf mode (2 k-planes per matmul,
           2 MACs/cell/cycle) -> ~2x tensor-engine throughput.
Validated in numpy vs the f64 reference: rel err ~3e-5 for both (incl. the
fp16 epilogue intermediates); gate is 2e-2.
"""

import math
import os
import sys

for _p in ("/opt/trn_rl_repo", "/root/.axon_site/_ro/trn_rl_repo"):
    if os.path.isdir(_p) and _p not in sys.path:
        sys.path.insert(0, _p)

import numpy as np
import ml_dtypes

import concourse.bacc as bacc
import concourse.tile as tile
from concourse import mybir
from concourse.bass_utils import run_bass_kernel_spmd

F32 = mybir.dt.float32
FP16 = mybir.dt.float16
AF = mybir.ActivationFunctionType
ALU = mybir.AluOpType

NCORES = 8
N = 1024
D = 1024
M = 10000
SLAB = 1250
SLABP = 1280
NPAD = SLABP - SLAB
JCHUNKS = [(0, 512), (512, 512), (1024, 256)]
NB_I = 8
SQRT30 = math.sqrt(30.0)

VARIANT = "fp8dr"  # "bf16" | "fp8dr"

_NC_CACHE = {}


def _build_nc(variant):
    nc = bacc.Bacc(
        "TRN2", target_bir_lowering=False, debug=False, num_devices=NCORES
    )
    DT = mybir.dt.float8e4 if variant == "fp8dr" else mybir.dt.bfloat16

    embD = nc.dram_tensor("embD", [128, 8 * 1024], DT, kind="ExternalInput").ap()
    slabD = nc.dram_tensor("slabD", [128, 8 * SLABP], DT, kind="ExternalInput").ap()
    labD = nc.dram_tensor("labD", [128, SLABP], FP16, kind="ExternalInput").ap()
    tgtD = nc.dram_tensor("tgtD", [128, NB_I], FP16, kind="ExternalInput").ap()
    out = nc.dram_tensor("out", [2, 128, NB_I], F32, kind="ExternalOutput").ap()

    with tile.TileContext(nc) as tc:
        with (
            tc.tile_pool(name="persist", bufs=1) as P,
            tc.tile_pool(name="emb", bufs=1) as PE,
            tc.tile_pool(name="slab", bufs=1) as PS,
            tc.tile_pool(name="work", bufs=3) as W,
            tc.tile_pool(name="psum_d", bufs=2, space="PSUM") as PP,
        ):
            biasn = P.tile([128, 1], F32)
            nc.vector.memset(biasn[:], -30.0)
            biasp = P.tile([128, 1], F32)
            nc.vector.memset(biasp[:], -44.8)
            scratch = P.tile([128, 1], F32)
            # pull the Exp LUT load off the critical path
            nc.scalar.activation(scratch[:], biasn[:], AF.Exp)

            labB = P.tile([128, SLABP], FP16)
            nc.gpsimd.dma_start(labB[:], labD[:])
            tgt_sb = P.tile([128, NB_I], FP16)
            nc.gpsimd.dma_start(tgt_sb[:], tgtD[:])

            embS = PE.tile([128, 8, 1024], DT)
            slabS = PS.tile([128, 8, SLABP], DT)
            # k-plane-pair-major arrival on the two HWDGE queues so the
            # first matmuls can start after ~2us (SWDGE desc-gen is slow)
            for t in range(4):
                s = 2 * t
                eq, sq = (nc.sync, nc.scalar) if t % 2 == 0 else (nc.scalar, nc.sync)
                eq.dma_start(
                    embS[:, s : s + 2, :], embD[:, s * 1024 : (s + 2) * 1024]
                )
                sq.dma_start(
                    slabS[:, s : s + 2, :], slabD[:, s * SLABP : (s + 2) * SLABP]
                )

            acc_n = P.tile([128, NB_I], F32)
            acc_p = P.tile([128, NB_I], F32)
            for ib in range(NB_I):
                i0 = ib * 128
                ps = PP.tile([128, 1536], F32, name="ps", tag="ps")
                if variant == "fp8dr":
                    for t in range(4):
                        lhs = embS[:, 2 * t : 2 * t + 2, i0 : i0 + 128]
                        for j0, jw in JCHUNKS:
                            nc.tensor.matmul(
                                ps[:, j0 : j0 + jw],
                                lhs,
                                slabS[:, 2 * t : 2 * t + 2, j0 : j0 + jw],
                                start=(t == 0),
                                stop=(t == 3),
                                perf_mode=mybir.MatmulPerfMode.DoubleRow,
                            )
                else:
                    for dd in range(8):
                        lhs = embS[:, dd, i0 : i0 + 128]
                        for j0, jw in JCHUNKS:
                            nc.tensor.matmul(
                                ps[:, j0 : j0 + jw],
                                lhs,
                                slabS[:, dd, j0 : j0 + jw],
                                start=(dd == 0),
                                stop=(dd == 7),
                            )
                g = ps[:, 0:SLABP]
                tgt_ib = tgt_sb[:, ib : ib + 1]
                # v = m - g  (DVE, psum-source); un^2 == (1-m)*v^2, so the
                # n-side square comes from vsq with one SBUF fp16 stt.
                v16 = W.tile([128, SLABP], FP16, name="v16", tag="v16")
                nc.vector.scalar_tensor_tensor(
                    v16[:], labB[:], tgt_ib, g, ALU.is_equal, ALU.subtract
                )
                vsq = W.tile([128, SLABP], FP16, name="vsq", tag="vsq")
                nc.vector.tensor_mul(vsq[:], v16[:], v16[:])
                usq = W.tile([128, SLABP], FP16, name="usq", tag="usq")
                nc.vector.scalar_tensor_tensor(
                    usq[:], labB[:], tgt_ib, vsq[:], ALU.not_equal, ALU.mult
                )
                # en = exp(30*usq - 30) ; ep = exp(30*vsq - 44.8)  (ACT, accum)
                en16 = W.tile([128, SLABP], FP16, name="en16", tag="en16")
                nc.scalar.activation(
                    en16[:],
                    usq[:],
                    AF.Exp,
                    bias=biasn[:, 0:1],
                    scale=30.0,
                    accum_out=acc_n[:, ib : ib + 1],
                )
                ep16 = W.tile([128, SLABP], FP16, name="ep16", tag="ep16")
                nc.scalar.activation(
                    ep16[:],
                    vsq[:],
                    AF.Exp,
                    bias=biasp[:, 0:1],
                    scale=30.0,
                    accum_out=acc_p[:, ib : ib + 1],
                )

            nc.sync.dma_start(out[0, :, :], acc_n[:])
            nc.sync.dma_start(out[1, :, :], acc_p[:])

    nc.compile()
    return nc


def _get_nc(variant=None):
    variant = variant or VARIANT
    if variant not in _NC_CACHE:
        _NC_CACHE[variant] = _build_nc(variant)
    return _NC_CACHE[variant]


def _prepare(embedding, old_cache_features, targets, old_cache_labels, variant=None):
    variant = variant or VARIANT
    np_dt = ml_dtypes.float8_e4m3 if variant == "fp8dr" else ml_dtypes.bfloat16

    emb = np.asarray(embedding, np.float32)
    oc = np.asarray(old_cache_features, np.float32)
    tg = np.asarray(targets, np.int64)
    ol = np.asarray(old_cache_labels, np.int64)

    embn = emb / np.linalg.norm(emb, axis=1, keepdims=True)
    cache = np.concatenate([embn, oc])[:M]
    labels = np.concatenate([tg, ol])[:M]

    cache_q = cache.astype(np_dt)
    embn_q = embn.astype(np_dt)
    # [128, 8, 1024] k-plane-major layout of embn.T
    embD = np.ascontiguousarray(
        embn_q.T.reshape(8, 128, N).transpose(1, 0, 2).reshape(128, 8 * N)
    )

    tgtC = np.ascontiguousarray(
        tg.reshape(NB_I, 128).T.astype(np.float16)
    )

    in_maps = []
    for k in range(NCORES):
        rows = cache_q[SLAB * k : SLAB * k + SLAB]  # [1250, D] quantized
        slabT = np.zeros((D, SLABP), np_dt)
        slabT[:, :SLAB] = rows.T
        slabD = np.ascontiguousarray(
            slabT.reshape(8, 128, SLABP).transpose(1, 0, 2).reshape(128, 8 * SLABP)
        )
        labs = np.full(SLABP, -1.0, np.float32)
        labs[:SLAB] = labels[SLAB * k : SLAB * k + SLAB]
        labB = np.ascontiguousarray(
            np.broadcast_to(labs.astype(np.float16), (128, SLABP))
        )
        in_maps.append(dict(embD=embD, slabD=slabD, labD=labB, tgtD=tgtC))

    # host-side corrections
    lab_counts = np.bincount(labels, minlength=1000)
    cnt = lab_counts[tg]  # per-row label matches incl. the diagonal
    gii = np.sum(embn_q.astype(np.float64) ** 2, axis=1)  # quantized diag sim
    aux = dict(cnt=cnt.astype(np.float64), gii=gii)
    return in_maps, aux


def _post(results, aux):
    sn = np.zeros(N, np.float64)
    sp = np.zeros(N, np.float64)
    for k in range(NCORES):
        o = np.asarray(results[k]["out"], np.float64)  # [2, 128, 8]
        sn += o[0].T.reshape(N)
        sp += o[1].T.reshape(N)
    sn -= (aux["cnt"] + NCORES * NPAD) * np.exp(-30.0)
    sp -= np.exp(30.0 * (1.0 - aux["gii"]) ** 2 - 44.8) + NCORES * NPAD * np.exp(
        -44.8
    )
    lse_n = 25.2 + np.log(np.maximum(sn, 1e-300))
    lse_p = 40.0 + np.log(np.maximum(sp, 1e-300))
    loss = np.mean(np.logaddexp(0.0, lse_p + lse_n))
    return np.float32(loss)


def _run(in_maps, variant=None, trace=False, **kwargs):
    nc = _get_nc(variant)
    return run_bass_kernel_spmd(
        nc, in_maps, core_ids=list(range(NCORES)), trace=trace, **kwargs
    )


def kernel(embedding, old_cache_features, targets, old_cache_labels):
    in_maps, aux = _prepare(
        embedding, old_cache_features, targets, old_cache_labels
    )
    res = _run(in_maps)
    return _post(res.results, aux)
